# revision 1
# baseline (speedup 1.0000x reference)
"""Trainium2 Bass kernel for nn_DecoderLayer_31086973288870.

Full decoder layer (QKV -> causal attention -> out-proj -> LN -> FFN -> LN),
S=2048, D=2048, 16 heads, INNER=8192, batch 1, fp32 reference.

Sharding (8 cores):
  - Attention: tensor-parallel over heads (2 heads/core). QKV column-parallel.
  - Out-proj: AllToAll turns per-core head-shards into per-core seq-shards
    (o^T chunks), then every core applies the FULL lin_w to its own 256-row
    seq slice. Only collective in the kernel: one 2MB AllToAll.
  - LN1/FFN/LN2: sequence-parallel. Every core holds the full (bf16) FFN
    weights and pushes only its own 256-row slice through them; weight DMA
    overlaps compute.
  - Host concatenates the 8 [256, 2048] output slices.

Dtypes: matmuls in float32r (full-rate fp32, ~1.5e-4 rel err) except the
attention-score matmul (bf16 q/k, errors land on ~0.2-magnitude logits) and
the FFN (bf16 weights to halve the 134MB weight stream). Accumulation is
always fp32 in PSUM; layernorms/softmax statistics in fp32.
"""

import math
import sys

import numpy as np

try:
    import concourse.bass as bass  # noqa: F401
except ImportError:  # pragma: no cover - harness containers stage it here
    sys.path.insert(0, "/opt/trn_rl_repo")
    import concourse.bass as bass  # noqa: F401

import ml_dtypes
import concourse.mybir as mybir
import concourse.tile as tile
from concourse import bacc
from concourse.bass_utils import run_bass_kernel_spmd
from concourse.masks import make_identity
from contextlib import ExitStack

S = 2048
D = 2048
HEADS = 16
HD = 128
INNER = 8192
NCORES = 8
HPC = HEADS // NCORES     # heads per core = 2
HDC = HPC * HD            # head dims per core = 256
SC = S // NCORES          # seq rows per core = 256
EPS = 1e-5
RSQ = 1.0 / math.sqrt(float(D))

f32 = mybir.dt.float32
FP = mybir.dt.float32r
bf16 = mybir.dt.bfloat16
AF = mybir.ActivationFunctionType
OP = mybir.AluOpType
AX = mybir.AxisListType

DEBUG = False


def _build(debug=DEBUG, nocc=False, rings="spread", w1bufs=3, qkpsbufs=3,
           vpsbufs=4, ptbufs=8, split_tr=False):
    nc = bacc.Bacc("TRN2", target_bir_lowering=False, debug=False,
                   num_devices=NCORES)

    def din(name, shape, dt):
        return nc.dram_tensor(name, shape, dt, kind="ExternalInput").ap()

    def dout(name, shape, dt):
        return nc.dram_tensor(name, shape, dt, kind="ExternalOutput").ap()

    xT_d = din("xT", [D, S], FP)
    xs_d = din("x_slice", [SC, D], f32)
    wq_d = din("wq", [D, HDC], FP)
    wk_d = din("wk", [D, HDC], FP)
    wv_d = din("wv", [D, HDC], FP)
    bq_d = din("bq", [HDC], f32)
    bk_d = din("bk", [HDC], f32)
    bv_d = din("bv", [HDC], FP)
    linw_d = din("lin_w", [D, D], FP)
    linb_d = din("lin_b", [D], FP)
    ff1_d = din("ff1_w", [D, INNER], bf16)
    ff1b_d = din("ff1_b", [INNER], f32)
    ff2_d = din("ff2_w", [INNER, D], bf16)
    ff2b_d = din("ff2_b", [D], f32)
    ln1g_d = din("ln1_g", [D], f32)
    ln1b_d = din("ln1_b", [D], f32)
    ln2g_d = din("ln2_g", [D], f32)
    ln2b_d = din("ln2_b", [D], f32)
    out_d = dout("out_slice", [SC, D], f32)

    if debug:
        dbg_q_d = dout("dbg_q", [HD, S], bf16)
        dbg_k_d = dout("dbg_k", [HD, S], bf16)
        dbg_v_d = dout("dbg_v", [HD, HDC], FP)
        dbg_oT_d = dout("dbg_oT", [HD, S], FP)
        dbg_u_d = dout("dbg_u", [HD, D], f32)
        dbg_h1_d = dout("dbg_h1", [HD, D], FP)
        dbg_gi_d = dout("dbg_gi", [HD, SC], bf16)

    if rings == "spread":
        wdma = nc.scalar.dma_start
        w2dma = nc.gpsimd.dma_start
    elif rings == "gp2":
        wdma = nc.sync.dma_start
        w2dma = nc.gpsimd.dma_start
    else:
        wdma = nc.sync.dma_start
        w2dma = nc.sync.dma_start

    with tile.TileContext(nc) as tc, ExitStack() as ctx:
        const = ctx.enter_context(tc.tile_pool(name="const", bufs=1))
        dram = ctx.enter_context(tc.tile_pool(name="dram", bufs=1, space="DRAM"))
        stat = ctx.enter_context(tc.tile_pool(name="stat", bufs=6))

        # f32r tiles must be produced by rounding instructions (DVE copy),
        # not memset, so build each in an f32 scratch then round-copy.
        ident_f = const.tile([128, 128], f32)
        make_identity(nc, ident_f[:])
        ident = const.tile([128, 128], FP)
        nc.vector.tensor_copy(ident[:], ident_f[:])
        onesf = const.tile([128, 128], f32)
        nc.gpsimd.memset(onesf[:], 1.0)
        ones_col = const.tile([128, 1], FP)
        nc.vector.tensor_copy(ones_col[:], onesf[:, 0:1])
        ones_row = const.tile([1, 128], FP)
        nc.vector.tensor_copy(ones_row[:], onesf[0:1, :])
        eps_sb = const.tile([128, 1], f32)
        nc.gpsimd.memset(eps_sb[:], EPS)
        # maskbig[i, u] = 1.0 iff u >= i + 384 else 0; slice [384-d : 896-d]
        # is the multiplicative "keep j >= i + delta" causal mask.
        maskf = const.tile([128, 896], f32)
        nc.gpsimd.memset(maskf[:], 1.0)
        nc.gpsimd.affine_select(
            out=maskf[:], in_=maskf[:], compare_op=OP.is_ge, fill=0.0,
            base=-384, channel_multiplier=-1, pattern=[[1, 896]])
        maskbig = const.tile([128, 896], FP)
        nc.vector.tensor_copy(maskbig[:], maskf[:])

        bq_sb = const.tile([128, HPC], f32)
        nc.sync.dma_start(bq_sb[:], bq_d.rearrange("(h p) -> p h", p=128))
        bk_sb = const.tile([128, HPC], f32)
        nc.sync.dma_start(bk_sb[:], bk_d.rearrange("(h p) -> p h", p=128))
        bv_sb = const.tile([1, HDC], FP)
        nc.sync.dma_start(bv_sb[:], bv_d[None, :])
        ff1b_sb = const.tile([128, INNER // 128], f32)
        nc.sync.dma_start(ff1b_sb[:], ff1b_d.rearrange("(t p) -> p t", p=128))

        def broadcast_row(pool, row_d, tag):
            """[D]-param from DRAM -> [128, D] SBUF broadcast tile."""
            t = pool.tile([128, D], f32, tag=tag, name=tag, bufs=1)
            nc.sync.dma_start(t[0:1, :], row_d[None, :])
            nc.gpsimd.partition_broadcast(t[:], t[0:1, :])
            return t

        def layernorm(u_tiles, g_row_d, b_row_d, scope, out_tiles, tag):
            G = broadcast_row(scope, g_row_d, f"G{tag}")
            B = broadcast_row(scope, b_row_d, f"B{tag}")
            for m, (ut, o) in enumerate(zip(u_tiles, out_tiles)):
                musum = stat.tile([128, 1], f32, tag="musum", name="musum")
                nc.vector.reduce_sum(musum[:], ut[:], axis=AX.X)
                sqsum = stat.tile([128, 1], f32, tag="sqsum", name="sqsum")
                scratch = scope.tile([128, D], f32, tag="ln_scratch",
                                     name="ln_scratch", bufs=2)
                nc.scalar.activation(scratch[:], ut[:], AF.Square,
                                     accum_out=sqsum[:])
                mu = stat.tile([128, 1], f32, tag="mu", name="mu")
                nc.vector.tensor_scalar(mu[:], musum[:], 1.0 / D, None, OP.mult)
                ex2 = stat.tile([128, 1], f32, tag="ex2", name="ex2")
                nc.vector.tensor_scalar(ex2[:], sqsum[:], 1.0 / D, None,
                                        OP.mult)
                mu2 = stat.tile([128, 1], f32, tag="mu2", name="mu2")
                nc.vector.tensor_tensor(mu2[:], mu[:], mu[:], OP.mult)
                var = stat.tile([128, 1], f32, tag="var", name="var")
                nc.vector.tensor_tensor(var[:], ex2[:], mu2[:], OP.subtract)
                std = stat.tile([128, 1], f32, tag="std", name="std")
                nc.scalar.activation(std[:], var[:], AF.Sqrt, bias=eps_sb[:])
                rstd = stat.tile([128, 1], f32, tag="rstd", name="rstd")
                nc.vector.reciprocal(rstd[:], std[:])
                nc.vector.tensor_scalar(o[:], ut[:], mu[:], rstd[:],
                                        OP.subtract, OP.mult)
                nc.vector.tensor_tensor(o[:], o[:], G[:], OP.mult)
                nc.vector.tensor_tensor(o[:], o[:], B[:], OP.add)

        # ---------------- Phase 1: QKV projections -----------------------
        qkv_keep = ExitStack()
        qk_pool = qkv_keep.enter_context(tc.tile_pool(name="qk", bufs=1))
        v_pool = qkv_keep.enter_context(tc.tile_pool(name="v", bufs=1))
        qT = []
        kT = []
        v_sb = []
        with tc.tile_pool(name="xT", bufs=1) as xp, \
             tc.tile_pool(name="qkv_ps", bufs=qkpsbufs, space="PSUM") as pp:
            xT_sb = []
            for i in range(16):
                t = xp.tile([128, S], FP, tag=f"xT{i}", name=f"xT{i}")
                nc.sync.dma_start(t[:], xT_d[i * 128:(i + 1) * 128, :])
                xT_sb.append(t)

            # v first (wv freed before the q/k weight pool opens)
            with tc.tile_pool(name="wv", bufs=1) as wvp:
                wv_sb = wvp.tile([128, 16, HDC], FP, tag="wv")
                wdma(
                    wv_sb[:], wv_d.rearrange("(k p) c -> p k c", p=128))
                for st in range(16):
                    ps = pp.tile([128, HDC], f32, tag="v_ps", name="v_ps", bufs=vpsbufs)
                    for kt in range(16):
                        nc.tensor.matmul(
                            ps[:], xT_sb[kt][:, st * 128:(st + 1) * 128],
                            wv_sb[:, kt, :], start=(kt == 0), stop=False)
                    nc.tensor.matmul(ps[:], ones_row[:], bv_sb[:],
                                     start=False, stop=True)
                    vt = v_pool.tile([128, HDC], FP, tag=f"v{st}",
                                     name=f"v{st}")
                    nc.vector.tensor_copy(vt[:], ps[:])
                    v_sb.append(vt)

            with tc.tile_pool(name="wqk", bufs=2) as wp:
                for (w_d, b_sb, dst_list, name) in (
                        (wq_d, bq_sb, qT, "q"), (wk_d, bk_sb, kT, "k")):
                    for h in range(HPC):
                        wt = wp.tile([128, 16, 128], FP, tag="w_qk")
                        wdma(
                            wt[:],
                            w_d[:, h * 128:(h + 1) * 128]
                            .rearrange("(k p) c -> p k c", p=128))
                        dst = qk_pool.tile([128, S], bf16, tag=f"{name}T{h}",
                                           name=f"{name}T{h}")
                        for qs in range(4):
                            ps = pp.tile([128, 512], f32, tag="qk_ps",
                                         name="qk_ps")
                            for kt in range(16):
                                nc.tensor.matmul(
                                    ps[:], wt[:, kt, :],
                                    xT_sb[kt][:, qs * 512:(qs + 1) * 512],
                                    start=(kt == 0), stop=(kt == 15))
                            nc.scalar.activation(
                                dst[:, qs * 512:(qs + 1) * 512], ps[:],
                                AF.Identity, bias=b_sb[:, h:h + 1])
                        dst_list.append(dst)

        # ---------------- Phase 2: causal attention ----------------------
        att_keep = ExitStack()
        ot_pool = att_keep.enter_context(tc.tile_pool(name="oT", bufs=1))
        oT = [ot_pool.tile([128, S], FP, tag=f"oT{h}", name=f"oT{h}")
              for h in range(HPC)]
        with tc.tile_pool(name="pT", bufs=ptbufs) as ptp, \
             tc.tile_pool(name="att_sm", bufs=4) as smp, \
             tc.tile_pool(name="att_ps", bufs=1, space="PSUM") as app:
            for h in range(HPC):
                for qs in range(4):
                    kmax = 4 * qs + 4
                    o_ps = app.tile([128, 512], f32, tag="o_ps", name="o_ps",
                                    bufs=2)
                    se_ps = app.tile([1, 512], f32, tag="se_ps", name="se_ps",
                                     bufs=2)
                    for kt in range(kmax):
                        s_ps = app.tile([128, 512], f32, tag="s_ps",
                                        name="s_ps", bufs=3)
                        nc.tensor.matmul(
                            s_ps[:], kT[h][:, kt * 128:(kt + 1) * 128],
                            qT[h][:, qs * 512:(qs + 1) * 512],
                            start=True, stop=True)
                        pt = ptp.tile([128, 512], FP, tag="pt", name="pt")
                        nc.scalar.activation(pt[:], s_ps[:], AF.Exp, scale=RSQ)
                        delta = kt * 128 - qs * 512
                        if delta >= 0:
                            nc.vector.tensor_tensor(
                                pt[:], pt[:],
                                maskbig[:, 384 - delta:896 - delta], OP.mult)
                        nc.tensor.matmul(
                            o_ps[:], v_sb[kt][:, h * 128:(h + 1) * 128],
                            pt[:], start=(kt == 0), stop=(kt == kmax - 1))
                        nc.tensor.matmul(
                            se_ps[:], ones_col[:], pt[:],
                            start=(kt == 0), stop=(kt == kmax - 1))
                    se_sb = smp.tile([1, 512], f32, tag="se_sb", name="se_sb")
                    nc.vector.tensor_copy(se_sb[:], se_ps[:])
                    rec = smp.tile([1, 512], f32, tag="rec", name="rec")
                    nc.vector.reciprocal(rec[:], se_sb[:])
                    bc = smp.tile([128, 512], f32, tag="bc", name="bc")
                    nc.gpsimd.partition_broadcast(bc[:], rec[:])
                    nc.vector.tensor_tensor(
                        oT[h][:, qs * 512:(qs + 1) * 512], o_ps[:], bc[:],
                        OP.mult)

        if debug:
            nc.sync.dma_start(dbg_q_d[:], qT[0][:])
            nc.sync.dma_start(dbg_k_d[:], kT[0][:])
            nc.sync.dma_start(dbg_v_d[:], v_sb[0][:])
            nc.sync.dma_start(dbg_oT_d[:], oT[0][:])

        # ---------------- Phase 3: AllToAll + output projection ----------
        a2a_in = dram.tile([NCORES, HDC, SC], FP)
        a2a_out = dram.tile([NCORES, HDC, SC], FP)
        for h in range(HPC):
            for c in range(NCORES):
                nc.sync.dma_start(
                    a2a_in[c, h * 128:(h + 1) * 128, :],
                    oT[h][:, c * SC:(c + 1) * SC])
        if nocc:
            # collective-free variant for single-core TimelineSim profiling
            nc.sync.dma_start(a2a_out[:], a2a_in[:])
        else:
            nc.gpsimd.collective_compute(
                "AllToAll", OP.bypass,
                replica_groups=[list(range(NCORES))],
                ins=[a2a_in[:]], outs=[a2a_out[:]])
        att_keep.close()   # oT dead once staged for the A2A
        qkv_keep.close()   # q/k/v dead after attention

        # Pools that outlive the next phases, in strict LIFO order:
        resA = ExitStack()
        res_pool = resA.enter_context(tc.tile_pool(name="res", bufs=1))
        h1b = [res_pool.tile([128, D], f32, tag=f"h1b{m}", name=f"h1b{m}")
               for m in range(2)]
        u2 = [res_pool.tile([128, D], f32, tag=f"u2{m}", name=f"u2{m}")
              for m in range(2)]
        h1T_keep = ExitStack()
        h1Tp = h1T_keep.enter_context(tc.tile_pool(name="h1T", bufs=1))
        h1_keep = ExitStack()
        h1p = h1_keep.enter_context(tc.tile_pool(name="h1", bufs=1))
        h1 = [h1p.tile([128, D], FP, tag=f"h1_{m}", name=f"h1_{m}")
              for m in range(2)]
        up_keep = ExitStack()
        up = up_keep.enter_context(tc.tile_pool(name="up", bufs=1))
        u_tiles = [up.tile([128, D], f32, tag=f"u{m}", name=f"u{m}")
                   for m in range(2)]
        xs_sb = []
        for m in range(2):
            t = up.tile([128, D], f32, tag=f"xs{m}", name=f"xs{m}")
            nc.sync.dma_start(t[:], xs_d[m * 128:(m + 1) * 128, :])
            xs_sb.append(t)

        with tc.tile_pool(name="linw", bufs=2) as lwp, \
             tc.tile_pool(name="ofT", bufs=1) as ofp, \
             tc.tile_pool(name="op_ps", bufs=2, space="PSUM") as opp:
            linb_sb = lwp.tile([1, D], FP, tag="linb", bufs=1)
            nc.sync.dma_start(linb_sb[:], linb_d[None, :])
            a2a_flat = a2a_out[:].rearrange("c p s -> (c p) s")
            ofT = []
            for kt in range(16):
                t = ofp.tile([128, SC], FP, tag=f"ofT{kt}", name=f"ofT{kt}")
                nc.sync.dma_start(t[:], a2a_flat[kt * 128:(kt + 1) * 128, :])
                ofT.append(t)
            for n in range(4):
                pss = [opp.tile([128, 512], f32, tag=f"op_ps{m}",
                                name=f"op_ps{m}") for m in range(2)]
                for half in range(2):
                    lwt = lwp.tile([128, 8, 512], FP, tag="lw")
                    wdma(
                        lwt[:],
                        linw_d[half * 1024:(half + 1) * 1024,
                               n * 512:(n + 1) * 512]
                        .rearrange("(k p) c -> p k c", p=128))
                    for m in range(2):
                        for k8 in range(8):
                            kt = half * 8 + k8
                            nc.tensor.matmul(
                                pss[m][:],
                                ofT[kt][:, m * 128:(m + 1) * 128],
                                lwt[:, k8, :],
                                start=(kt == 0), stop=False)
                for m in range(2):
                    nc.tensor.matmul(pss[m][:], ones_row[:],
                                     linb_sb[:, n * 512:(n + 1) * 512],
                                     start=False, stop=True)
                    nc.vector.tensor_tensor(
                        u_tiles[m][:, n * 512:(n + 1) * 512], pss[m][:],
                        xs_sb[m][:, n * 512:(n + 1) * 512], OP.add)

        # ---------------- Phase 4: LN1 + transpose + residual base -------
        with tc.tile_pool(name="ln1p", bufs=1) as lnp, \
             tc.tile_pool(name="tr_ps", bufs=2, space="PSUM") as tpp:
            layernorm(u_tiles, ln1g_d, ln1b_d, lnp, h1, "1")
            if debug:
                nc.sync.dma_start(dbg_u_d[:], u_tiles[0][:])
                nc.sync.dma_start(dbg_h1_d[:], h1[0][:])
            h1T = [h1Tp.tile([128, SC], bf16, tag=f"h1T{kt}",
                             name=f"h1T{kt}") for kt in range(16)]
            with tc.tile_pool(name="tr_ps", bufs=3, space="PSUM") as tpp:
                mk = ([(m, kt) for m in range(2) for kt in range(16)]
                      if split_tr else
                      [(m, kt) for kt in range(16) for m in range(2)])
                for m, kt in mk:
                    tp = tpp.tile([128, 128], FP, tag="tr_ps", name="tr_ps")
                    nc.tensor.transpose(
                        tp[:], h1[m][:, kt * 128:(kt + 1) * 128], ident[:])
                    nc.vector.tensor_copy(
                        h1T[kt][:, m * 128:(m + 1) * 128], tp[:])
            B2f = broadcast_row(lnp, ff2b_d, "B2f")
            for m in range(2):
                nc.vector.tensor_tensor(h1b[m][:], h1[m][:], B2f[:], OP.add)
        up_keep.close()
        h1_keep.close()

        # ---------------- Phase 5: FFN (sequence-parallel) ---------------
        with tc.tile_pool(name="gi", bufs=1) as gip, \
             tc.tile_pool(name="w1", bufs=w1bufs) as w1p, \
             tc.tile_pool(name="w2", bufs=2) as w2p, \
             tc.tile_pool(name="ffn_ps", bufs=2, space="PSUM") as fpp:
            ginner = []
            for ib in range(16):
                w1t = w1p.tile([128, 16, 512], bf16, tag="w1")
                wdma(
                    w1t[:],
                    ff1_d[:, ib * 512:(ib + 1) * 512]
                    .rearrange("(k p) c -> p k c", p=128))
                for ms in range(4):
                    it = ib * 4 + ms
                    ps = fpp.tile([128, SC], f32, tag="f1_ps", name="f1_ps")
                    for kt in range(16):
                        nc.tensor.matmul(
                            ps[:], w1t[:, kt, ms * 128:(ms + 1) * 128],
                            h1T[kt][:], start=(kt == 0), stop=(kt == 15))
                    g = gip.tile([128, SC], bf16, tag=f"gi{it}", name=f"gi{it}")
                    nc.scalar.activation(g[:], ps[:], AF.Gelu,
                                         bias=ff1b_sb[:, it:it + 1])
                    ginner.append(g)
            if debug:
                nc.sync.dma_start(dbg_gi_d[:], ginner[0][:])

            for n in range(4):
                pss = [fpp.tile([128, 512], f32, tag=f"f2_ps{m}",
                                name=f"f2ps{m}") for m in range(2)]
                for ktc in range(4):
                    w2t = w2p.tile([128, 16, 512], bf16, tag="w2")
                    w2dma(
                        w2t[:],
                        ff2_d[ktc * 2048:(ktc + 1) * 2048,
                              n * 512:(n + 1) * 512]
                        .rearrange("(k p) c -> p k c", p=128))
                    for m in range(2):
                        for k2 in range(16):
                            kt = ktc * 16 + k2
                            nc.tensor.matmul(
                                pss[m][:],
                                ginner[kt][:, m * 128:(m + 1) * 128],
                                w2t[:, k2, :],
                                start=(kt == 0), stop=(kt == 63))
                for m in range(2):
                    nc.vector.tensor_tensor(
                        u2[m][:, n * 512:(n + 1) * 512], pss[m][:],
                        h1b[m][:, n * 512:(n + 1) * 512], OP.add)

            # ------------ Phase 6: LN2 (in-place on u2) + store ----------
            layernorm(u2, ln2g_d, ln2b_d, w1p, u2, "2")
            for m in range(2):
                nc.sync.dma_start(out_d[m * 128:(m + 1) * 128, :], u2[m][:])
        h1T_keep.close()
        resA.close()

    nc.compile()
    return nc


_NC_CACHE = {}


def _get_nc(debug=DEBUG, nocc=False, **kw):
    key = (debug, nocc, tuple(sorted(kw.items())))
    if key not in _NC_CACHE:
        _NC_CACHE[key] = _build(debug, nocc, **kw)
    return _NC_CACHE[key]


def make_in_maps(x, C_w, C_b, lin_w, lin_b, ff1_w, ff1_b, ff2_w, ff2_b,
                 ln1_g, ln1_b, ln2_g, ln2_b):
    x2 = np.asarray(x, dtype=np.float32)[0]            # [S, D]
    xT = np.ascontiguousarray(x2.T)                    # [D, S]
    C_w = np.asarray(C_w, dtype=np.float32)
    C_b = np.asarray(C_b, dtype=np.float32)
    ff1_bf = np.asarray(ff1_w).astype(ml_dtypes.bfloat16)
    ff2_bf = np.asarray(ff2_w).astype(ml_dtypes.bfloat16)
    lin_w = np.ascontiguousarray(np.asarray(lin_w, dtype=np.float32))
    common = {
        "xT": xT,
        "lin_w": lin_w,
        "lin_b": np.asarray(lin_b, dtype=np.float32),
        "ff1_w": ff1_bf,
        "ff1_b": np.asarray(ff1_b, dtype=np.float32),
        "ff2_w": ff2_bf,
        "ff2_b": np.asarray(ff2_b, dtype=np.float32),
        "ln1_g": np.asarray(ln1_g, dtype=np.float32),
        "ln1_b": np.asarray(ln1_b, dtype=np.float32),
        "ln2_g": np.asarray(ln2_g, dtype=np.float32),
        "ln2_b": np.asarray(ln2_b, dtype=np.float32),
    }
    in_maps = []
    for c in range(NCORES):
        sl = slice(c * HDC, (c + 1) * HDC)
        m = dict(common)
        m["wq"] = np.ascontiguousarray(C_w[:, sl])
        m["wk"] = np.ascontiguousarray(C_w[:, D:][:, sl])
        m["wv"] = np.ascontiguousarray(C_w[:, 2 * D:][:, sl])
        m["bq"] = np.ascontiguousarray(C_b[sl])
        m["bk"] = np.ascontiguousarray(C_b[D:][sl])
        m["bv"] = np.ascontiguousarray(C_b[2 * D:][sl])
        m["x_slice"] = np.ascontiguousarray(x2[c * SC:(c + 1) * SC, :])
        in_maps.append(m)
    return in_maps


def run(in_maps, debug=DEBUG):
    nc = _get_nc(debug)
    return run_bass_kernel_spmd(nc, in_maps, list(range(NCORES)))


def kernel(**inputs):
    in_maps = make_in_maps(**inputs)
    res = run(in_maps)
    out = np.concatenate(
        [res.results[c]["out_slice"] for c in range(NCORES)], axis=0)
    return out.reshape(1, S, D).astype(np.float32)



# revision 14
# speedup vs baseline: 1.0899x; 1.0899x over previous
"""Trainium2 Bass kernel for nn_DecoderLayer_31086973288870.

Full decoder layer (QKV -> causal attention -> out-proj -> LN -> FFN -> LN),
S=2048, D=2048, 16 heads, INNER=8192, batch 1, fp32 reference.

Sharding (8 cores):
  - Attention: tensor-parallel over heads (2 heads/core). QKV column-parallel.
  - Out-proj: per-head AllToAlls turn per-core head-shards into per-core
    seq-shards (o^T chunks); each core applies the FULL lin_w to its own
    256-row seq slice. Two 512KB bf16 AllToAlls; the first overlaps with
    head-1 attention compute.
  - LN1/FFN/LN2: sequence-parallel. Every core holds the full (bf16) FFN
    weights and pushes only its own 256-row slice through them; weight DMA
    is prefetched/streamed on otherwise-idle queues so it overlaps compute.
  - ln1_g/ln1_b are folded into ff1_w/ff1_b host-side so the FFN input
    transpose depends only on the normalized (pre-affine) activations.
  - Host concatenates the 8 [256, 2048] output slices.

All matmuls run in bf16 (fp32 PSUM accumulation); layernorm/softmax
statistics and residuals stay fp32.
"""

import math
import sys

import numpy as np

try:
    import concourse.bass as bass  # noqa: F401
except ImportError:  # pragma: no cover - harness containers stage it here
    sys.path.insert(0, "/opt/trn_rl_repo")
    import concourse.bass as bass  # noqa: F401

import ml_dtypes
import concourse.mybir as mybir
import concourse.tile as tile
from concourse import bacc
from concourse.bass_utils import run_bass_kernel_spmd
from concourse.masks import make_identity
from contextlib import ExitStack

S = 2048
D = 2048
HEADS = 16
HD = 128
INNER = 8192
NCORES = 8
HPC = HEADS // NCORES     # heads per core = 2
HDC = HPC * HD            # head dims per core = 256
SC = S // NCORES          # seq rows per core = 256
EPS = 1e-5
RSQ = 1.0 / math.sqrt(float(D))

f32 = mybir.dt.float32
FP = mybir.dt.float32r
bf16 = mybir.dt.bfloat16
AF = mybir.ActivationFunctionType
OP = mybir.AluOpType
AX = mybir.AxisListType


def _build(nocc=False, w1bufs=2, w2bufs=2):
    nc = bacc.Bacc("TRN2", target_bir_lowering=False, debug=False,
                   num_devices=NCORES)

    def din(name, shape, dt):
        return nc.dram_tensor(name, shape, dt, kind="ExternalInput").ap()

    def dout(name, shape, dt):
        return nc.dram_tensor(name, shape, dt, kind="ExternalOutput").ap()

    xT_d = din("xT", [D, S], bf16)
    xs_d = din("x_slice", [SC, D], f32)
    wq_d = din("wq", [D, HDC], bf16)
    wk_d = din("wk", [D, HDC], bf16)
    wv_d = din("wv", [D, HDC], bf16)
    bq_d = din("bq", [HDC], f32)
    bk_d = din("bk", [HDC], f32)
    bv_d = din("bv", [HDC], bf16)
    linw_d = din("lin_w", [D, D], bf16)
    linb_d = din("lin_b", [D], bf16)
    ff1_d = din("ff1_w", [D, INNER], bf16)       # pre-scaled by ln1_g
    ff1b_d = din("ff1_b", [INNER], f32)          # includes ln1_b @ ff1_w
    ff2_d = din("ff2_w", [INNER, D], bf16)
    lnffb_d = din("lnff_b", [D], f32)            # ln1_b + ff2_b
    ln1g_d = din("ln1_g", [D], f32)
    ln2g_d = din("ln2_g", [D], f32)
    ln2b_d = din("ln2_b", [D], f32)
    out_d = dout("out_slice", [SC, D], f32)

    with tile.TileContext(nc) as tc, ExitStack() as ctx:
        const = ctx.enter_context(tc.tile_pool(name="const", bufs=1))
        dram = ctx.enter_context(tc.tile_pool(name="dram", bufs=1, space="DRAM"))
        stat = ctx.enter_context(tc.tile_pool(name="stat", bufs=6))
        res = ctx.enter_context(tc.tile_pool(name="res", bufs=1))

        # ---- constants ----
        ident_f = const.tile([128, 128], f32)
        make_identity(nc, ident_f[:])
        ones_col = const.tile([128, 1], bf16)
        nc.gpsimd.memset(ones_col[:], 1.0)
        ones_row = const.tile([1, 128], bf16)
        nc.gpsimd.memset(ones_row[:], 1.0)
        eps_sb = const.tile([128, 1], f32)
        nc.gpsimd.memset(eps_sb[:], EPS)
        # maskbig[i, u] = 1.0 iff u >= i + 384 else 0; slice [384-d : 896-d]
        # is the multiplicative "keep j >= i + delta" causal mask.
        maskbig = const.tile([128, 896], bf16)
        nc.gpsimd.memset(maskbig[:], 1.0)
        nc.gpsimd.affine_select(
            out=maskbig[:], in_=maskbig[:], compare_op=OP.is_ge, fill=0.0,
            base=-384, channel_multiplier=-1, pattern=[[1, 896]])

        # persistent fp32 tiles: residual-2 accumulators + h1T (bf16)
        u2 = [res.tile([128, D], f32, tag=f"u2{m}", name=f"u2{m}")
              for m in range(2)]
        h1b = [res.tile([128, D], f32, tag=f"h1b{m}", name=f"h1b{m}")
               for m in range(2)]
        h1T = res.tile([128, 16, SC], bf16, tag="h1T", name="h1T")

        def broadcast_row(pool, row_d, tag):
            """[D]-param from DRAM -> [128, D] SBUF broadcast tile."""
            t = pool.tile([128, D], f32, tag=tag, name=tag, bufs=1)
            nc.sync.dma_start(t[0:1, :], row_d[None, :])
            nc.gpsimd.partition_broadcast(t[:], t[0:1, :])
            return t

        def layernorm_z(u_tiles, scope, out_tiles):
            """Normalize (no affine): out = (u - mu) * rsqrt(var + eps).
            Yields m after each tile's normalize so PE work can interleave."""
            for m in range(2):
                ut, o = u_tiles[m], out_tiles[m]
                musum = stat.tile([128, 1], f32, tag="musum", name="musum")
                nc.vector.reduce_sum(musum[:], ut[:], axis=AX.X)
                sqsum = stat.tile([128, 1], f32, tag="sqsum", name="sqsum")
                scratch = scope.tile([128, D], f32, tag="ln_scratch",
                                     name="ln_scratch", bufs=1)
                nc.scalar.activation(scratch[:], ut[:], AF.Square,
                                     accum_out=sqsum[:])
                mu = stat.tile([128, 1], f32, tag="mu", name="mu")
                nc.vector.tensor_scalar(mu[:], musum[:], 1.0 / D, None, OP.mult)
                ex2 = stat.tile([128, 1], f32, tag="ex2", name="ex2")
                nc.vector.tensor_scalar(ex2[:], sqsum[:], 1.0 / D, None,
                                        OP.mult)
                mu2 = stat.tile([128, 1], f32, tag="mu2", name="mu2")
                nc.vector.tensor_tensor(mu2[:], mu[:], mu[:], OP.mult)
                var = stat.tile([128, 1], f32, tag="var", name="var")
                nc.vector.tensor_tensor(var[:], ex2[:], mu2[:], OP.subtract)
                std = stat.tile([128, 1], f32, tag="std", name="std")
                nc.scalar.activation(std[:], var[:], AF.Sqrt, bias=eps_sb[:])
                rstd = stat.tile([128, 1], f32, tag="rstd", name="rstd")
                nc.vector.reciprocal(rstd[:], std[:])
                nc.vector.tensor_scalar(o[:], ut[:], mu[:], rstd[:],
                                        OP.subtract, OP.mult)
                yield m

        # ff1 stream pool on the RIGHT side; first w1bufs tiles are
        # prefetched during attention (gated on phase-1 completion).
        w1p = ctx.enter_context(tc.tile_pool(name="w1", bufs=w1bufs,
                                             side="right"))

        # ---------------- Phase 1: QKV projections -----------------------
        qkv_keep = ExitStack()
        qk_pool = qkv_keep.enter_context(tc.tile_pool(name="qk", bufs=1))
        v_pool = qkv_keep.enter_context(tc.tile_pool(name="v", bufs=1))
        qT = []
        kT = []
        v_sb = []
        with tc.tile_pool(name="xT", bufs=1) as xp, \
             tc.tile_pool(name="wv", bufs=1) as wvp, \
             tc.tile_pool(name="wqk", bufs=2) as wp, \
             tc.tile_pool(name="qkv_ps", bufs=1, space="PSUM") as pp:
            # wv chunks interleaved with xT chunks in issue order so the
            # kt-ordered v matmuls can start as soon as chunk 0 lands.
            wv_sb = wvp.tile([128, 16, HDC], bf16, tag="wv")
            wv_r = wv_d.rearrange("(k p) c -> p k c", p=128)
            xT_sb = [xp.tile([128, 4, S], bf16, tag=f"xT{i}", name=f"xT{i}")
                     for i in range(4)]
            for i in range(4):
                nc.sync.dma_start(wv_sb[:, 4 * i:4 * (i + 1), :],
                                  wv_r[:, 4 * i:4 * (i + 1), :])
                eng = nc.sync if i % 2 == 0 else nc.scalar
                eng.dma_start(
                    xT_sb[i][:], xT_d[i * 512:(i + 1) * 512, :]
                    .rearrange("(k p) s -> p k s", p=128))

            def xTs(kt):
                return xT_sb[kt // 4][:, kt % 4, :]
            bq_sb = const.tile([128, HPC], f32)
            nc.sync.dma_start(bq_sb[:], bq_d.rearrange("(h p) -> p h", p=128))
            bk_sb = const.tile([128, HPC], f32)
            nc.sync.dma_start(bk_sb[:], bk_d.rearrange("(h p) -> p h", p=128))
            bv_sb = const.tile([1, HDC], bf16)
            nc.sync.dma_start(bv_sb[:], bv_d[None, :])
            linb_sb = const.tile([1, D], bf16)
            nc.sync.dma_start(linb_sb[:], linb_d[None, :])
            ff1b_sb = const.tile([128, INNER // 128], f32)
            nc.sync.dma_start(ff1b_sb[:],
                              ff1b_d.rearrange("(t p) -> p t", p=128))


            # v: kt-outer in two half-passes so matmuls trickle as xT lands;
            # two seq-chunks share one [128, 512] PSUM bank.
            for half in range(2):
                pss = [pp.tile([128, 2 * HDC], f32, tag=f"v_ps{i}",
                               name=f"v_ps{i}", bufs=1) for i in range(4)]
                for kt in range(16):
                    for st8 in range(8):
                        st = half * 8 + st8
                        ps = pss[st8 // 2][:, (st8 % 2) * HDC:
                                           (st8 % 2 + 1) * HDC]
                        # start=True clears has_written for the WHOLE bank:
                        # only the bank's first matmul may set it. The odd
                        # half still overwrites on its first matmul because
                        # its has_written bits are clear.
                        nc.tensor.matmul(
                            ps, xTs(kt)[:, st * 128:(st + 1) * 128],
                            wv_sb[:, kt, :],
                            start=(kt == 0 and st8 % 2 == 0), stop=False)
                for st8 in range(8):
                    st = half * 8 + st8
                    ps = pss[st8 // 2][:, (st8 % 2) * HDC:
                                       (st8 % 2 + 1) * HDC]
                    nc.tensor.matmul(ps, ones_row[:], bv_sb[:],
                                     start=False, stop=True)
                    vt = v_pool.tile([128, HDC], bf16, tag=f"v{st}",
                                     name=f"v{st}")
                    nc.scalar.activation(vt[:], ps, AF.Identity)
                    v_sb.append(vt)

            for (w_d, b_sb, dst_list, name) in (
                    (wq_d, bq_sb, qT, "q"), (wk_d, bk_sb, kT, "k")):
                for h in range(HPC):
                    wt = wp.tile([128, 16, 128], bf16, tag="w_qk")
                    nc.sync.dma_start(
                        wt[:],
                        w_d[:, h * 128:(h + 1) * 128]
                        .rearrange("(k p) c -> p k c", p=128))
                    dst = qk_pool.tile([128, S], bf16, tag=f"{name}T{h}",
                                       name=f"{name}T{h}")
                    for qs in range(4):
                        ps = pp.tile([128, 512], f32, tag="qk_ps",
                                     name="qk_ps", bufs=3)
                        for kt in range(16):
                            nc.tensor.matmul(
                                ps[:], wt[:, kt, :],
                                xTs(kt)[:, qs * 512:(qs + 1) * 512],
                                start=(kt == 0), stop=(kt == 15))
                        nc.scalar.activation(
                            dst[:, qs * 512:(qs + 1) * 512], ps[:],
                            AF.Identity, bias=b_sb[:, h:h + 1])
                    dst_list.append(dst)

        # -------- prefetch issue point (gated on phase-1 completion) -----
        # A tiny Pool-engine op reading the last v tile keeps the prefetch
        # DMAs queued behind phase 1, so they don't steal xT bandwidth.
        late_keep = ExitStack()
        w2p = late_keep.enter_context(tc.tile_pool(name="w2", bufs=w2bufs,
                                                   side="right"))
        lwp = late_keep.enter_context(tc.tile_pool(name="linw", bufs=2,
                                                   side="right"))
        gate = stat.tile([1, HDC], bf16, tag="gate", name="gate")
        nc.gpsimd.tensor_copy(gate[:], v_sb[15][0:1, :])
        w1_tiles = {}
        for ib in range(w1bufs):
            t = w1p.tile([128, 16, 512], bf16, tag="w1")
            nc.gpsimd.dma_start(
                t[:], ff1_d[:, ib * 512:(ib + 1) * 512]
                .rearrange("(k p) c -> p k c", p=128))
            w1_tiles[ib] = t
        lw_tiles = {}
        for n in range(2):
            t = lwp.tile([128, 16, 512], bf16, tag="lw")
            nc.gpsimd.dma_start(
                t[:], linw_d[:, n * 512:(n + 1) * 512]
                .rearrange("(k p) c -> p k c", p=128))
            lw_tiles[n] = t
        w2_tiles = {}
        for idx in range(w2bufs):
            n, ktc = idx // 4, idx % 4
            t = w2p.tile([128, 16, 512], bf16, tag="w2")
            nc.gpsimd.dma_start(
                t[:], ff2_d[ktc * 2048:(ktc + 1) * 2048,
                            n * 512:(n + 1) * 512]
                .rearrange("(k p) c -> p k c", p=128))
            w2_tiles[idx] = t

        # ---------------- Phase 2: causal attention ----------------------
        a2a_in = [dram.tile([NCORES, HD, SC], bf16, tag=f"a2a_in{h}",
                            name=f"a2a_in{h}") for h in range(HPC)]
        a2a_out = [dram.tile([NCORES, HD, SC], bf16, tag=f"a2a_out{h}",
                             name=f"a2a_out{h}") for h in range(HPC)]
        with tc.tile_pool(name="oT", bufs=1) as ot_pool, \
             tc.tile_pool(name="pT", bufs=8) as ptp, \
             tc.tile_pool(name="att_sm", bufs=2) as smp, \
             tc.tile_pool(name="att_ps", bufs=1, space="PSUM") as app:
            for h in range(HPC):
                oTh = ot_pool.tile([128, S], bf16, tag=f"oT{h}", name=f"oT{h}")
                for qs in range(4):
                    kmax = 4 * qs + 4
                    o_ps = app.tile([128, 512], f32, tag="o_ps", name="o_ps",
                                    bufs=2)
                    se_ps = app.tile([1, 512], f32, tag="se_ps", name="se_ps",
                                     bufs=2)
                    for kt in range(kmax):
                        s_ps = app.tile([128, 512], f32, tag="s_ps",
                                        name="s_ps", bufs=3)
                        nc.tensor.matmul(
                            s_ps[:], kT[h][:, kt * 128:(kt + 1) * 128],
                            qT[h][:, qs * 512:(qs + 1) * 512],
                            start=True, stop=True)
                        pt = ptp.tile([128, 512], bf16, tag="pt", name="pt")
                        nc.scalar.activation(pt[:], s_ps[:], AF.Exp, scale=RSQ)
                        delta = kt * 128 - qs * 512
                        if delta >= 0:
                            nc.vector.tensor_tensor(
                                pt[:], pt[:],
                                maskbig[:, 384 - delta:896 - delta], OP.mult)
                        nc.tensor.matmul(
                            o_ps[:], v_sb[kt][:, h * 128:(h + 1) * 128],
                            pt[:], start=(kt == 0), stop=(kt == kmax - 1))
                        nc.tensor.matmul(
                            se_ps[:], ones_col[:], pt[:],
                            start=(kt == 0), stop=(kt == kmax - 1))
                    se_sb = smp.tile([1, 512], f32, tag="se_sb", name="se_sb")
                    nc.vector.tensor_copy(se_sb[:], se_ps[:])
                    rec = smp.tile([1, 512], f32, tag="rec", name="rec")
                    nc.vector.reciprocal(rec[:], se_sb[:])
                    bc = smp.tile([128, 512], f32, tag="bc", name="bc")
                    nc.gpsimd.partition_broadcast(bc[:], rec[:])
                    nc.vector.tensor_tensor(
                        oTh[:, qs * 512:(qs + 1) * 512], o_ps[:], bc[:],
                        OP.mult)
                    # stage this qs-slice into the AllToAll send buffer
                    for j in range(2):
                        nc.sync.dma_start(
                            a2a_in[h][2 * qs + j, :, :],
                            oTh[:, qs * 512 + j * SC:
                                qs * 512 + (j + 1) * SC])
                # per-head AllToAll: head 0's collective overlaps head 1
                if nocc:
                    nc.sync.dma_start(a2a_out[h][:], a2a_in[h][:])
                else:
                    nc.gpsimd.collective_compute(
                        "AllToAll", OP.bypass,
                        replica_groups=[list(range(NCORES))],
                        ins=[a2a_in[h][:]], outs=[a2a_out[h][:]])
        qkv_keep.close()   # q/k/v dead after attention

        # ---------------- Phase 3: output projection ---------------------
        up_keep = ExitStack()
        up = up_keep.enter_context(tc.tile_pool(name="up", bufs=1))
        xs_sb = []
        for m in range(2):
            t = up.tile([128, D], f32, tag=f"xs{m}", name=f"xs{m}")
            nc.sync.dma_start(t[:], xs_d[m * 128:(m + 1) * 128, :])
            xs_sb.append(t)

        with tc.tile_pool(name="ofT", bufs=1) as ofp, \
             tc.tile_pool(name="op_ps", bufs=2, space="PSUM") as opp:
            # ofT[kt] holds attention-output dims [kt*128, (kt+1)*128) of
            # the core's seq slice; kt = c*2 + h.
            ofT = ofp.tile([128, 16, SC], bf16, tag="ofT", name="ofT")
            for h in range(HPC):
                nc.sync.dma_start(
                    ofT[:].rearrange("p (c h2) s -> p c h2 s", h2=2)
                    [:, :, h, :],
                    a2a_out[h][:].rearrange("c p s -> p c s"))
            for n in range(4):
                if n >= 2:
                    lwt = lwp.tile([128, 16, 512], bf16, tag="lw")
                    nc.scalar.dma_start(
                        lwt[:],
                        linw_d[:, n * 512:(n + 1) * 512]
                        .rearrange("(k p) c -> p k c", p=128))
                    lw_tiles[n] = lwt
                lwt = lw_tiles[n]
                pss = [opp.tile([128, 512], f32, tag=f"op_ps{m}",
                                name=f"op_ps{m}") for m in range(2)]
                # h0's kt (even) first, then h1's: all of h0's matmuls can
                # run before the second collective lands
                kts = [c * 2 + h for h in range(2) for c in range(8)]
                for ki, kt in enumerate(kts):
                    for m in range(2):
                        nc.tensor.matmul(
                            pss[m][:], ofT[:, kt, m * 128:(m + 1) * 128],
                            lwt[:, kt, :], start=(ki == 0), stop=False)
                for m in range(2):
                    nc.tensor.matmul(pss[m][:], ones_row[:],
                                     linb_sb[:, n * 512:(n + 1) * 512],
                                     start=False, stop=True)
                    # u written in-place into xs tile (residual add)
                    nc.vector.tensor_tensor(
                        xs_sb[m][:, n * 512:(n + 1) * 512], pss[m][:],
                        xs_sb[m][:, n * 512:(n + 1) * 512], OP.add)

        # ---------- Phase 4: LN1 (z only, in-place) + transpose ----------
        # h1T gets the *pre-affine* z (ln1_g/ln1_b are folded into ff1_w/b
        # host-side); the residual branch h1b = z*ln1_g + (ln1_b + ff2_b)
        # runs on DVE off the FF1 critical path.
        with tc.tile_pool(name="lnp", bufs=1) as lnp, \
             tc.tile_pool(name="tr_ps", bufs=3, space="PSUM") as tpp:
            G1 = broadcast_row(lnp, ln1g_d, "G1")
            BF = broadcast_row(lnp, lnffb_d, "BF")
            for m in layernorm_z(xs_sb, lnp, xs_sb):
                zm = xs_sb[m]
                for kt in range(16):
                    tp = tpp.tile([128, 128], f32, tag="tr_ps", name="tr_ps")
                    nc.tensor.transpose(
                        tp[:], zm[:, kt * 128:(kt + 1) * 128], ident_f[:])
                    nc.vector.tensor_copy(
                        h1T[:, kt, m * 128:(m + 1) * 128], tp[:])
                nc.vector.tensor_tensor(h1b[m][:], zm[:], G1[:], OP.mult)
                nc.vector.tensor_tensor(h1b[m][:], h1b[m][:], BF[:], OP.add)
        up_keep.close()

        # ---------------- Phase 5: FFN (sequence-parallel) ---------------
        with tc.tile_pool(name="gi", bufs=1) as gip, \
             tc.tile_pool(name="ffn_ps", bufs=2, space="PSUM") as fpp:
            ginner = []
            for ib in range(16):
                if ib in w1_tiles:
                    w1t = w1_tiles[ib]
                else:
                    w1t = w1p.tile([128, 16, 512], bf16, tag="w1")
                    nc.scalar.dma_start(
                        w1t[:],
                        ff1_d[:, ib * 512:(ib + 1) * 512]
                        .rearrange("(k p) c -> p k c", p=128))
                for ms in range(4):
                    it = ib * 4 + ms
                    ps = fpp.tile([128, SC], f32, tag="f1_ps", name="f1_ps")
                    for kt in range(16):
                        nc.tensor.matmul(
                            ps[:], w1t[:, kt, ms * 128:(ms + 1) * 128],
                            h1T[:, kt, :], start=(kt == 0), stop=(kt == 15))
                    g = gip.tile([128, SC], bf16, tag=f"gi{it}", name=f"gi{it}")
                    nc.scalar.activation(g[:], ps[:], AF.Gelu,
                                         bias=ff1b_sb[:, it:it + 1])
                    ginner.append(g)

            for n in range(4):
                pss = [fpp.tile([128, 512], f32, tag=f"f2_ps{m}",
                                name=f"f2ps{m}") for m in range(2)]
                for ktc in range(4):
                    idx = n * 4 + ktc
                    # pre-issue next chunk's DMA so it overlaps this one
                    nxt = idx + 1
                    if w2bufs <= nxt < 16:
                        n2, k2c = nxt // 4, nxt % 4
                        t = w2p.tile([128, 16, 512], bf16, tag="w2")
                        nc.sync.dma_start(
                            t[:], ff2_d[k2c * 2048:(k2c + 1) * 2048,
                                        n2 * 512:(n2 + 1) * 512]
                            .rearrange("(k p) c -> p k c", p=128))
                        w2_tiles[nxt] = t
                    w2t = w2_tiles[idx]
                    for m in range(2):
                        for k2 in range(16):
                            kt = ktc * 16 + k2
                            nc.tensor.matmul(
                                pss[m][:],
                                ginner[kt][:, m * 128:(m + 1) * 128],
                                w2t[:, k2, :],
                                start=(kt == 0), stop=(kt == 63))
                for m in range(2):
                    nc.vector.tensor_tensor(
                        u2[m][:, n * 512:(n + 1) * 512], pss[m][:],
                        h1b[m][:, n * 512:(n + 1) * 512], OP.add)
            late_keep.close()

            # ------------ Phase 6: LN2 (in-place on u2) + store ----------
            G2 = broadcast_row(gip, ln2g_d, "G2")
            B2 = broadcast_row(gip, ln2b_d, "B2")
            for m in layernorm_z(u2, gip, u2):
                nc.vector.tensor_tensor(u2[m][:], u2[m][:], G2[:], OP.mult)
                nc.vector.tensor_tensor(u2[m][:], u2[m][:], B2[:], OP.add)
                nc.sync.dma_start(out_d[m * 128:(m + 1) * 128, :], u2[m][:])

    nc.compile()
    return nc


_NC_CACHE = {}


def _get_nc(debug=False, nocc=False, **kw):
    key = (nocc, tuple(sorted(kw.items())))
    if key not in _NC_CACHE:
        _NC_CACHE[key] = _build(nocc, **kw)
    return _NC_CACHE[key]


def make_in_maps(x, C_w, C_b, lin_w, lin_b, ff1_w, ff1_b, ff2_w, ff2_b,
                 ln1_g, ln1_b, ln2_g, ln2_b):
    bf = ml_dtypes.bfloat16
    x2 = np.asarray(x, dtype=np.float32)[0]            # [S, D]
    xT = np.ascontiguousarray(x2.T).astype(bf)         # [D, S] bf16
    C_w = np.asarray(C_w, dtype=np.float32)
    C_b = np.asarray(C_b, dtype=np.float32)
    # fold LN1's affine into ff1: gelu((z*g+b) @ W1 + b1)
    #   = gelu(z @ (g[:,None]*W1) + (b1 + b @ W1))
    g64 = np.asarray(ln1_g, np.float64)
    b64 = np.asarray(ln1_b, np.float64)
    W1 = np.asarray(ff1_w, np.float64)
    ff1_scaled = (g64[:, None] * W1).astype(bf)
    ff1b_adj = (np.asarray(ff1_b, np.float64) + b64 @ W1).astype(np.float32)
    lnff_b = (b64 + np.asarray(ff2_b, np.float64)).astype(np.float32)
    common = {
        "xT": xT,
        "lin_w": np.ascontiguousarray(np.asarray(lin_w)).astype(bf),
        "lin_b": np.asarray(lin_b).astype(bf),
        "ff1_w": ff1_scaled,
        "ff1_b": ff1b_adj,
        "ff2_w": np.asarray(ff2_w).astype(bf),
        "lnff_b": lnff_b,
        "ln1_g": np.asarray(ln1_g, dtype=np.float32),
        "ln2_g": np.asarray(ln2_g, dtype=np.float32),
        "ln2_b": np.asarray(ln2_b, dtype=np.float32),
    }
    in_maps = []
    for c in range(NCORES):
        sl = slice(c * HDC, (c + 1) * HDC)
        m = dict(common)
        m["wq"] = np.ascontiguousarray(C_w[:, sl]).astype(bf)
        m["wk"] = np.ascontiguousarray(C_w[:, D:][:, sl]).astype(bf)
        m["wv"] = np.ascontiguousarray(C_w[:, 2 * D:][:, sl]).astype(bf)
        m["bq"] = np.ascontiguousarray(C_b[sl])
        m["bk"] = np.ascontiguousarray(C_b[D:][sl])
        m["bv"] = np.ascontiguousarray(C_b[2 * D:][sl]).astype(bf)
        m["x_slice"] = np.ascontiguousarray(x2[c * SC:(c + 1) * SC, :])
        in_maps.append(m)
    return in_maps


def run(in_maps, debug=False):
    nc = _get_nc(debug)
    return run_bass_kernel_spmd(nc, in_maps, list(range(NCORES)))


def kernel(**inputs):
    in_maps = make_in_maps(**inputs)
    res = run(in_maps)
    out = np.concatenate(
        [res.results[c]["out_slice"] for c in range(NCORES)], axis=0)
    return out.reshape(1, S, D).astype(np.float32)
